# revision 46
# baseline (speedup 1.0000x reference)
"""Trainium2 Bass kernel for nn_DeepConv1d (self-contained).

Math (per batch b):
  xr   = linear-interp(deep, 1024 -> 4096)           # commutes with 1x1 conv
  y    = conv_w @ xr + conv_b                        # == interp(conv_w @ deep + conv_b)
  xs   = GAMA*(y-mean)/(var_unbiased+EPS)            # per-channel over n
  loss_k[c,l] = sech^2(xs_pad[c,l+k]-xs_pad[c,l+3])  # k=0..6, reflect pad 3
  S    = sum_k loss_k ;  W_k = (loss_k/S)*x_pad[:,l+k]
  out[o,l] = sum_{c,k} fc_w[o, 7c+k] * W_k[c,l]

On-chip identities:
  - interp(conv(.)) == conv(interp(.)); interp via first differences D.
  - sum(y^2) computed analytically from ys and D (no pass over y):
      sum y^2 = 4*sum ys^2 + sum ys*(D[j+1]-D[j]) + 0.3125*sum D^2
  - sech^2(d) = 4*sigmoid(2d)*sigmoid(-2d); the normalization scale
    f = GAMA/(var+EPS) folds into the sigmoid's per-partition scale.
  - lv = sigmoid(z)*sigmoid(-z) = sigmoid'(z): two ACT sigmoids and one
    2x DVE mul (cheaper on the bottleneck DVE than a 1x STT).
  - S/4 = 0.25 + sum sigmoid' accumulated ON THE PE: 7 matmuls with an
    identity lhsT (6 shifted lv views + a 0.25 const tile) into PSUM,
    then one DVE reciprocal_approx_fast gives G4 = 4/S directly.
  - loss_k arrays are shifted views of 3 gap arrays lv_g (g = |k-3|).

Layout: 2 batches per core packed on 128 partitions (64 channels each).
The gap arrays are processed in 4 column-quarters aligned with the four
l-chunks so each chunk's G4/GL/W/GEMM pipeline starts as soon as its
quarter's sigmoids land.  GEMM is issued k-outer so each fck slice loads
once and PE follows the DVE W-mul stream with minimal lag.  GPSIMD is
unused for compute: measured SBUF contention makes concurrent GPSIMD
cost DVE ~2x more than GPSIMD contributes.
"""
import contextlib

import numpy as np
import ml_dtypes

import concourse.bass as bass
import concourse.bacc as bacc_mod
import concourse.mybir as mybir
import concourse.tile as tile
from concourse.bass_utils import run_bass_kernel_spmd

bf16 = ml_dtypes.bfloat16
AF = mybir.ActivationFunctionType
ALU = mybir.AluOpType

KS = 7
PAD = 3
GAMA = 0.5
EPS = 1e-9
N = 4096
ND = 1024
NP = N + 2 * PAD       # 4102
L3 = N + PAD           # 4099: gap array length
NCORES = 8
NCH = 4                # l-chunks
CW = N // NCH          # 1024
SB = [0, 1027, 2051, 3075, 4099]   # gap-array quarter bounds

F32 = mybir.dt.float32
BF = mybir.dt.bfloat16


def kernel_body(tc, xp_d, cwdp_d, cb_d, fck_d, eyen_d, out_d):
    nc = tc.nc

    ctx = contextlib.ExitStack()
    with ctx:
        io = ctx.enter_context(tc.tile_pool(name="io", bufs=1))
        mid = ctx.enter_context(tc.tile_pool(name="mid", bufs=1))
        loss = ctx.enter_context(tc.tile_pool(name="loss", bufs=1))
        sap = ctx.enter_context(tc.tile_pool(name="sap", bufs=2))
        ck = ctx.enter_context(tc.tile_pool(name="ck", bufs=2))
        stp = ctx.enter_context(tc.tile_pool(name="stp", bufs=2))
        msq = ctx.enter_context(tc.tile_pool(name="msq", bufs=2, space="PSUM"))
        ppa = ctx.enter_context(tc.tile_pool(name="ppa", bufs=3, space="PSUM"))

        # ---------------- input DMAs (small first) ----------------
        cwdp = io.tile([32, 128 + ND], F32, tag="cwdp")
        nc.sync.dma_start(out=cwdp, in_=cwdp_d[:, :])
        cb = io.tile([128, 1], F32, tag="cb")
        nc.sync.dma_start(out=cb, in_=cb_d[:, :])
        fck = io.tile([128, KS, 128], BF, tag="fck")
        nc.sync.dma_start(out=fck, in_=fck_d[:, :, :])
        eyen = io.tile([128, 128], BF, tag="eyen")
        nc.sync.dma_start(out=eyen, in_=eyen_d[:, :])
        xp = io.tile([128, NP], BF, tag="xp")          # x reflect-padded
        xs1 = io.tile([128, NP - 1], BF, tag="xs1")    # same, shifted 1 elem
        nc.sync.dma_start(out=xp, in_=xp_d[:, :])
        nc.sync.dma_start(out=xs1, in_=xp_d[:, 1:NP])

        warm = mid.tile([128, 1], F32, tag="warm")
        nc.scalar.activation(out=warm, in_=cb, func=AF.Sigmoid, scale=1.0)
        negq = mid.tile([128, CW], BF, tag="negq")
        nc.vector.memset(negq, 0.25)

        cw = cwdp[:, 0:128]
        dp = cwdp[:, 128:128 + ND]

        # ---------------- conv (PE) + bias (ACT) ------------
        ys_ps = ppa.tile([128, ND], F32, tag="acc", name="conv")
        for h in range(2):
            nc.tensor.matmul(
                out=ys_ps[:, h * 512:(h + 1) * 512],
                lhsT=cw,
                rhs=dp[:, h * 512:(h + 1) * 512],
                start=True, stop=True,
            )
        # bias-ACT in halves so the Dp diff can start ~1.5us earlier
        ysb = mid.tile([128, ND], BF, tag="ysb")
        sumh = [mid.tile([128, 1], F32, tag=f"sumh{h}", name=f"sumh{h}")
                for h in range(2)]
        for h in range(2):
            nc.scalar.activation(out=ysb[:, h * 512:(h + 1) * 512],
                                 in_=ys_ps[:, h * 512:(h + 1) * 512],
                                 func=AF.Identity, bias=cb, scale=1.0,
                                 accum_out=sumh[h])
        sumys = mid.tile([128, 1], F32, tag="sumys")
        nc.vector.tensor_add(out=sumys, in0=sumh[0], in1=sumh[1])
        # ---------------- stats pieces on DVE (bias cancels in diffs) ----
        Dp = mid.tile([128, ND + 1], BF, tag="Dp")
        nc.vector.memset(Dp[:, 0:1], 0.0)
        nc.vector.memset(Dp[:, ND:ND + 1], 0.0)
        nc.vector.tensor_sub(out=Dp[:, 1:ND], in0=ysb[:, 1:ND],
                             in1=ysb[:, 0:ND - 1])

        # -------- unit diffs u of the padded interpolated signal ---------
        # y is piecewise linear: its unit diffs d[4j+r] are periodic combos
        # of D: d = gam_r*D_j + del_r*D_{j+1}, gam=[.25,.125,0,0],
        # del=[0,.125,.25,.25].  The reflect pad makes the 3 edge diffs
        # negated mirrors.  dy1 = u (a view!), dy2b[l]=u[l+1]+u[l+2],
        # dy3[l]=u[l]+dy2b[l] -- so ypad itself is never materialized.
        gam4 = mid.tile([128, 4], BF, tag="gam4")
        for i, v in enumerate([0.25, 0.125, 0.0, 0.0]):
            nc.vector.memset(gam4[:, i:i + 1], v)
        del4 = mid.tile([128, 4], BF, tag="del4")
        for i, v in enumerate([0.0, 0.125, 0.25, 0.25]):
            nc.vector.memset(del4[:, i:i + 1], v)

        def _rep4(ap_, off, j0, nj):
            return bass.AP(tensor=ap_.tensor, offset=ap_.offset + off + j0,
                           ap=[list(ap_.ap[0]), [1, nj], [0, 4]])

        def _bcast4(ap_, nj):
            return bass.AP(tensor=ap_.tensor, offset=ap_.offset,
                           ap=[list(ap_.ap[0]), [0, nj], [1, 4]])

        def _rev3(ap_, last):
            return bass.AP(tensor=ap_.tensor, offset=ap_.offset + last,
                           ap=[list(ap_.ap[0]), [-1, 3]])

        u = mid.tile([128, 4101], BF, tag="u")
        At = mid.tile([128, 2048], BF, tag="At")
        Bt = mid.tile([128, 2048], BF, tag="Bt")  # to trim
        HJ = ND // 2
        # sum((ys+cb)^2) without touching y: ACT Square straight off PSUM
        sy2 = mid.tile([128, 1], F32, tag="sy2")
        dump = mid.tile([128, ND + 1], F32, tag="dump", name="dump_a")
        nc.scalar.activation(out=dump[:, 0:ND], in_=ys_ps, func=AF.Square,
                             bias=cb, scale=1.0, accum_out=sy2)

        def u_h0(piece):
            # h0 in two pieces so dyq0/sigq0 unblock after 257 j's
            j0, nj = (0, 257) if piece == 0 else (257, 255)
            a4 = At[:, 4 * j0:4 * (j0 + nj)].rearrange(
                "p (j r) -> p j r", r=4)
            b4 = Bt[:, 4 * j0:4 * (j0 + nj)].rearrange(
                "p (j r) -> p j r", r=4)
            nc.vector.tensor_mul(out=a4, in0=_rep4(Dp, 0, j0, nj),
                                 in1=_bcast4(gam4, nj))
            nc.vector.tensor_mul(out=b4, in0=_rep4(Dp, 1, j0, nj),
                                 in1=_bcast4(del4, nj))
            nc.vector.tensor_add(out=u[:, 3 + 4 * j0:3 + 4 * (j0 + nj)],
                                 in0=At[:, 4 * j0:4 * (j0 + nj)],
                                 in1=Bt[:, 4 * j0:4 * (j0 + nj)])
            if piece == 0:   # u[0:3] = -[u[5],u[4],u[3]]
                nc.vector.tensor_scalar_mul(out=u[:, 0:3], in0=_rev3(u, 5),
                                            scalar1=-1.0)

        def u_half(h):
            j0 = h * HJ
            a4 = At.rearrange("p (j r) -> p j r", r=4)
            b4 = Bt.rearrange("p (j r) -> p j r", r=4)
            nc.vector.tensor_mul(out=a4, in0=Dp4[:, 0:HJ, :],
                                 in1=_bcast4(gam4, HJ))
            nc.vector.tensor_mul(out=b4, in0=Dp4[:, 1:1 + HJ, :],
                                 in1=_bcast4(del4, HJ))
            nc.vector.tensor_add(out=u[:, 3 + j0 * 4:3 + (j0 + HJ) * 4],
                                 in0=At, in1=Bt)
            # u[4098:4101] = -[u[4097],u[4096],u[4095]]
            nc.vector.tensor_scalar_mul(out=u[:, 4098:4101],
                                        in0=_rev3(u, 4097), scalar1=-1.0)

        # remaining stats: sum D^2 (ACT), cross = sum ysb*ddif (DVE STT)
        sd2 = mid.tile([128, 1], F32, tag="sd2")
        dump2 = mid.tile([128, ND + 1], F32, tag="dump", name="dump_b")
        nc.scalar.activation(out=dump2, in_=Dp, func=AF.Square, accum_out=sd2)
        ddif = mid.tile([128, ND], BF, tag="ddif")
        nc.vector.tensor_sub(out=ddif, in0=Dp[:, 1:ND + 1], in1=Dp[:, 0:ND])
        junk = mid.tile([128, 2048], BF, tag="Bt", name="junkb")
        cross = mid.tile([128, 1], F32, tag="cross")
        nc.vector.scalar_tensor_tensor(
            out=junk[:, 0:ND], in0=ddif, scalar=1.0, in1=ysb,
            op0=ALU.mult, op1=ALU.mult, accum_out=cross)

        # sum_y = 4*sumys exactly; sum_y2 = 4*sy2 + cross + 0.3125*sd2
        sum_y = mid.tile([128, 1], F32, tag="sum_y")
        nc.vector.tensor_scalar_mul(out=sum_y, in0=sumys, scalar1=4.0)
        e1 = mid.tile([128, 1], F32, tag="e1")
        nc.vector.tensor_scalar_mul(out=e1, in0=sy2, scalar1=4.0)
        e2 = mid.tile([128, 1], F32, tag="e2")
        nc.vector.tensor_scalar(out=e2, in0=sd2, scalar1=0.3125,
                                scalar2=0.0, op0=ALU.mult, op1=ALU.add)
        e3 = mid.tile([128, 1], F32, tag="e3")
        nc.vector.tensor_add(out=e3, in0=e1, in1=e2)
        sum_y2 = mid.tile([128, 1], F32, tag="sum_y2")
        nc.vector.tensor_add(out=sum_y2, in0=e3, in1=cross)
        # mean = sum_y/N; var = (sum_y2 - sum_y*mean)/(N-1); f = GAMA/(var+EPS)
        mean = mid.tile([128, 1], F32, tag="mean")
        nc.vector.tensor_scalar_mul(out=mean, in0=sum_y, scalar1=1.0 / N)
        t0 = mid.tile([128, 1], F32, tag="t0")
        nc.vector.tensor_mul(out=t0, in0=sum_y, in1=mean)
        t2 = mid.tile([128, 1], F32, tag="t2")
        nc.vector.tensor_sub(out=t2, in0=sum_y2, in1=t0)
        denom = mid.tile([128, 1], F32, tag="denom")
        nc.vector.tensor_scalar(out=denom, in0=t2, scalar1=1.0 / (N - 1),
                                scalar2=EPS, op0=ALU.mult, op1=ALU.add)
        inv = mid.tile([128, 1], F32, tag="inv")
        nc.vector.reciprocal(out=inv, in_=denom)
        f2p = mid.tile([128, 1], F32, tag="f2p")
        nc.vector.tensor_scalar_mul(out=f2p, in0=inv, scalar1=2.0 * GAMA)
        f2n = mid.tile([128, 1], F32, tag="f2n")
        nc.vector.tensor_scalar_mul(out=f2n, in0=inv, scalar1=-2.0 * GAMA)

        # ACT replicates Dp 4x into a packed table so the coefficient
        # muls run in 2x mode (a stride-0 DVE operand would force 1x)
        Dp4 = mid.tile([128, 513, 4], BF, tag="Dp4")   # covers j in [512,1024]
        for r in range(4):
            dst = bass.AP(tensor=Dp4.tensor, offset=Dp4.offset + r,
                          ap=[list(Dp4.ap[0]), [4, 513]])
            nc.scalar.copy(out=dst, in_=Dp[:, 512:ND + 1])
        u_h0(0)

        # ---------------- gap diffs (bf16 2x), quarters ------------------
        lv1 = loss.tile([128, L3], BF, tag="lv1")
        lv2b = loss.tile([128, L3], BF, tag="lv2b")
        lv3 = loss.tile([128, L3], BF, tag="lv3")
        P12 = loss.tile([128, N], BF, tag="P12")
        P21 = loss.tile([128, N], BF, tag="P21")
        P30 = loss.tile([128, N], BF, tag="P30")
        G4 = loss.tile([128, N], BF, tag="G4")

        dy2b = loss.tile([128, L3], BF, tag="dy2b")
        dy3 = loss.tile([128, L3], BF, tag="dy3")
        sa_tiles = {}

        def dy2b_q(q):
            a, b = SB[q], SB[q + 1]
            nc.vector.tensor_add(out=dy2b[:, a:b], in0=u[:, 1 + a:1 + b],
                                 in1=u[:, 2 + a:2 + b])

        def dy3_q(q):
            a, b = SB[q], SB[q + 1]
            nc.vector.tensor_add(out=dy3[:, a:b], in0=u[:, a:b],
                                 in1=dy2b[:, a:b])

        def dyq(q):
            dy2b_q(q)
            dy3_q(q)

        def sigq(q):
            a, b = SB[q], SB[q + 1]
            for g, (srct, nm) in enumerate(
                    [(dy3, "s3"), (dy2b, "s2"), (u, "s1")]):
                for sgn, scl in [("a", f2p), ("b", f2n)]:
                    sa = sap.tile([128, 1027], BF, tag=nm + sgn,
                                  name=f"{nm}{sgn}_{q}")
                    sa_tiles[(nm + sgn, q)] = sa
                    nc.scalar.activation(out=sa[:, 0:b - a], in_=srct[:, a:b],
                                         func=AF.Sigmoid, scale=scl)
            return

        def lvpq(q, filler=None):
            a, b = SB[q], SB[q + 1]
            # P product column ranges per quarter (aligned to x index = l)
            pa, pb = CW * q, CW * (q + 1)
            gaps = [(lv3, P30, xp, 0, 0, "s3"), (lv2b, P21, xs1, 0, 0, "s2"),
                    (lv1, P12, xp, 2, 2, "s1")]
            fl = list(filler) if filler else []
            for (lv, P, xin, xoff, poff, nm) in gaps:
                sa = sa_tiles[(nm + "a", q)]
                sb = sa_tiles[(nm + "b", q)]
                nc.vector.tensor_mul(out=lv[:, a:b], in0=sa[:, 0:b - a],
                                     in1=sb[:, 0:b - a])
                if fl:
                    fl.pop(0)()
            # P written chunk-aligned; lv/x read at +poff/+xoff shifts
            # (SB bounds chosen so lv[pb-1+poff] is inside quarter q)
            for (lv, P, xin, xoff, poff, nm) in gaps:
                nc.vector.tensor_mul(out=P[:, pa:pb],
                                     in0=lv[:, pa + poff:pb + poff],
                                     in1=xin[:, pa + xoff:pb + xoff])

        # per-chunk compute
        def chunk(c, lo=None, cwid=CW):
            if lo is None:
                lo = c * CW
            qtiles = []
            for s in range(cwid // 512):
                cs = lo + s * 512
                q_ps = msq.tile([128, 512], F32, tag="q", name=f"q_{c}_{s}")
                qtiles.append(q_ps)
                views = [
                    negq[:, 0:512],
                    lv3[:, cs:cs + 512], lv3[:, cs + 3:cs + 515],
                    lv2b[:, cs:cs + 512], lv2b[:, cs + 2:cs + 514],
                    lv1[:, cs + 2:cs + 514], lv1[:, cs + 3:cs + 515],
                ]
                for vi, v in enumerate(views):
                    nc.tensor.matmul(out=q_ps, lhsT=eyen, rhs=v,
                                     start=(vi == 0),
                                     stop=(vi == len(views) - 1))
            for s in range(cwid // 512):
                cs = lo + s * 512
                g32 = ck.tile([128, 512], F32, tag="g32", name=f"g32_{c}_{s}")
                nc.vector.reciprocal_approx_fast(out=g32, in_=qtiles[s])
                if c == NCH - 1:
                    nc.vector.tensor_scalar_mul(out=G4[:, cs:cs + 512],
                                                in0=g32, scalar1=1.0)
                else:
                    nc.scalar.copy(out=G4[:, cs:cs + 512], in_=g32)

            GL1 = ck.tile([128, cwid], BF, tag="GL1", name=f"GL1_{c}")
            GL2 = ck.tile([128, cwid], BF, tag="GL2", name=f"GL2_{c}")
            GL3 = ck.tile([128, cwid], BF, tag="GL3", name=f"GL3_{c}")
            W = [ck.tile([128, cwid], BF, tag=f"W{k}", name=f"W{k}_{c}")
                 for k in range(KS)]
            accs = [ppa.tile([128, cwid], F32, tag="acc", name=f"acc_{c}_{b}")
                    for b in range(2)]

            border = (1, 0) if c >= NCH - 1 else (0, 1)

            def mm(k):
                for b in border:
                    prow = slice(64 * b, 64 * (b + 1))
                    for sub in range(cwid // 512):
                        cs2 = slice(sub * 512, (sub + 1) * 512)
                        nc.tensor.matmul(
                            out=accs[b][:, cs2],
                            lhsT=fck[prow, k, :],
                            rhs=W[k][prow, cs2],
                            start=(k == 0), stop=(k == KS - 1),
                        )

            # W muls in k order; GEMM matmuls follow each W
            nc.vector.tensor_mul(out=W[0], in0=G4[:, lo:lo + cwid],
                                 in1=P30[:, lo:lo + cwid])
            mm(0)
            nc.vector.tensor_mul(out=W[1], in0=G4[:, lo:lo + cwid],
                                 in1=P21[:, lo:lo + cwid])
            mm(1)
            nc.vector.tensor_mul(out=W[2], in0=G4[:, lo:lo + cwid],
                                 in1=P12[:, lo:lo + cwid])
            mm(2)
            nc.vector.tensor_mul(out=W[3], in0=G4[:, lo:lo + cwid],
                                 in1=xs1[:, lo + 2:lo + 2 + cwid])
            mm(3)
            nc.vector.tensor_mul(out=GL1, in0=lv1[:, lo + 3:lo + 3 + cwid],
                                 in1=G4[:, lo:lo + cwid])
            nc.vector.tensor_mul(out=W[4], in0=GL1, in1=xp[:, lo + 4:lo + 4 + cwid])
            mm(4)
            nc.vector.tensor_mul(out=GL2, in0=lv2b[:, lo + 2:lo + 2 + cwid],
                                 in1=G4[:, lo:lo + cwid])
            nc.vector.tensor_mul(out=W[5], in0=GL2, in1=xs1[:, lo + 4:lo + 4 + cwid])
            mm(5)
            nc.vector.tensor_mul(out=GL3, in0=lv3[:, lo + 3:lo + 3 + cwid],
                                 in1=G4[:, lo:lo + cwid])
            nc.vector.tensor_mul(out=W[6], in0=GL3, in1=xp[:, lo + 6:lo + 6 + cwid])
            mm(6)

            return accs

        def stages(c, accs, lo=None, cwid=CW):
            if lo is None:
                lo = c * CW
            if c < NCH - 1:
                for b in range(2):
                    stage = stp.tile([128, 1024], BF, tag="stage",
                                     name=f"stage_{c}_{b}")
                    nc.scalar.copy(out=stage, in_=accs[b])
                    nc.sync.dma_start(out=out_d[:, b, lo:lo + CW], in_=stage)
            else:
                for b in range(2):
                    stage = stp.tile([128, 1024], BF, tag="stage",
                                     name=f"stage_{c}_{b}")
                    for s in range(cwid // 512):
                        sl = slice(s * 512, (s + 1) * 512)
                        if b == 1:
                            nc.vector.tensor_copy(out=stage[:, sl],
                                                  in_=accs[b][:, sl])
                        else:
                            nc.scalar.copy(out=stage[:, sl], in_=accs[b][:, sl])
                        nc.sync.dma_start(
                            out=out_d[:, b, lo + s * 512:lo + (s + 1) * 512],
                            in_=stage[:, sl])

        # ---------------- pipeline: quarters drive chunks ----------------
        dyq(0)
        sigq(0)
        u_h0(1)
        u_half(1)
        dyq(1)
        lvpq(0)
        a0 = chunk(0)
        sigq(1)
        lvpq(1, filler=[lambda: dy2b_q(2), lambda: dy3_q(2)])
        a1 = chunk(1)
        sigq(2)
        stages(0, a0)
        lvpq(2, filler=[lambda: dy2b_q(3), lambda: dy3_q(3)])
        a2 = chunk(2)
        sigq(3)
        stages(1, a1)
        lvpq(3)
        a3a = chunk(3, lo=3072, cwid=512)
        stages(2, a2)
        a3b = chunk(4, lo=3584, cwid=512)
        stages(3, a3a, lo=3072, cwid=512)
        stages(4, a3b, lo=3584, cwid=512)


def build_nc():
    nc = bacc_mod.Bacc(None, target_bir_lowering=False)
    xp_d = nc.dram_tensor("xp", [128, NP], BF, kind="ExternalInput")
    cwdp_d = nc.dram_tensor("cwdp", [32, 128 + ND], F32, kind="ExternalInput")
    cb_d = nc.dram_tensor("cb", [128, 1], F32, kind="ExternalInput")
    fck_d = nc.dram_tensor("fck", [128, KS, 128], BF, kind="ExternalInput")
    eyen_d = nc.dram_tensor("eyen", [128, 128], BF, kind="ExternalInput")
    out_d = nc.dram_tensor("out", [128, 2, N], BF, kind="ExternalOutput")
    with tile.TileContext(nc) as tc:
        kernel_body(tc, xp_d, cwdp_d, cb_d, fck_d, eyen_d, out_d)
    nc.compile()
    return nc


def prep_inputs(deep, x, conv_w, conv_b, fc_w):
    deep = np.asarray(deep, np.float32)
    x = np.asarray(x, np.float32)
    conv_w = np.asarray(conv_w, np.float32)
    conv_b = np.asarray(conv_b, np.float32)
    fc_w = np.asarray(fc_w, np.float32)

    xpad = np.pad(x, ((0, 0), (0, 0), (PAD, PAD)), mode="reflect")
    xp_all = np.ascontiguousarray(xpad.reshape(NCORES, 128, NP)).astype(bf16)
    dp_all = np.ascontiguousarray(deep.reshape(NCORES, 32, ND))
    cw_blk = np.zeros((32, 128), np.float32)
    cw_blk[0:16, 0:64] = conv_w.T
    cw_blk[16:32, 64:128] = conv_w.T
    cb = np.ascontiguousarray(
        np.concatenate([conv_b, conv_b]).reshape(128, 1).astype(np.float32))
    fc3 = fc_w.reshape(128, 64, KS)
    fck_half = np.transpose(fc3, (1, 2, 0)).copy()
    fck_half[:, PAD, :] *= 0.25   # W_3 = G4*x = 4*(G*x), not lv-scaled
    fck = np.ascontiguousarray(
        np.concatenate([fck_half, fck_half], axis=0)).astype(bf16)
    eyen = np.ascontiguousarray(np.eye(128, dtype=np.float32).astype(bf16))
    return [
        {"xp": np.ascontiguousarray(xp_all[ci]),
         "cwdp": np.ascontiguousarray(
             np.concatenate([cw_blk, dp_all[ci]], axis=1)),
         "cb": cb, "fck": fck, "eyen": eyen}
        for ci in range(NCORES)
    ]


def gather_out(results):
    out_full = np.empty((16, 128, N), np.float32)
    for ci in range(NCORES):
        o = results[ci]["out"]
        out_full[2 * ci] = o[:, 0].astype(np.float32)
        out_full[2 * ci + 1] = o[:, 1].astype(np.float32)
    return out_full


_CACHED = {}


def _get_nc():
    if "nc" not in _CACHED:
        _CACHED["nc"] = build_nc()
    return _CACHED["nc"]


def kernel(deep, x, conv_w, conv_b, fc_w):
    in_maps = prep_inputs(deep, x, conv_w, conv_b, fc_w)
    nc = _get_nc()
    res = run_bass_kernel_spmd(nc, in_maps, core_ids=list(range(NCORES)))
    return gather_out(res.results)
